# revision 2
# baseline (speedup 1.0000x reference)
"""BioRNN Trainium2 kernel — v2 (rescaled psum + decay-matrix drain).

Per-core math (batch-sharded 8-way, B=8 per core), T steps:
    h_t = 0.8 h_{t-1} + r_t,  r_t = relu(alpha*(z_t + h_{t-1} @ W))
with W = w_eff, z_t = x_t @ w_in + noise_t (+ b_rec folded into noise).

Key restructurings vs v1:
  * K-block exponential rescaling: psum accumulates
        Q_j = Q_{j-1} + rt_{t-1} @ W,   rt = 0.8^{-(j+1)} * r  (fp16)
    in place (no per-step 0.8*psum scaling, no identity matmuls, no ACT
    muls on the critical path). Block boundary every K=32 steps: ACT
    rescale S = 0.8^K * Q -> 4 identity matmuls re-inject (bank reset).
  * relu: custom DVE op  rt = relu(C0*Q + C1*zt)  with host-prescaled
    zt = alpha*0.8^{-(j+1)} * z (x and noise scaled on host). C0 = 0.25.
    Split a (chunks 0,1 of psum) / b (chunks 2,3) for latency overlap.
  * No per-step h update at all: the ring stores rt directly; h is
    reconstructed at drain time per 128-step block entirely on the PE:
        h_blk = Dcur^T @ rtT_blk + Dtail^T @ rtT_prev_blk
    where Dcur/Dtail are constant decay matrices (contributions older
    than 256 steps are < 0.8^128 ~ 4e-13, negligible).

Engine budget per step: PE 16 matmuls (~420ns), DVE 2 custom relus,
ACT only boundary/drain copies. Critical path: psum commit -> DVE relu
-> sbuf -> next matmuls.
"""

import numpy as np
from contextlib import ExitStack

import concourse.bass as bass
import concourse.mybir as mybir
import concourse.tile as tile
from concourse import bacc
from concourse import dve_ops
from concourse.dve_spec import Spec, Src0, Src1, C0, C1, relu as _relu_expr, lower
from concourse.dve_uop import DveOpSpec
from concourse.masks import make_identity


def _register_relu_sc2():
    """Register fused out = relu(in0*c0 + in1*c1) custom DVE op (idempotent)."""
    name = "RELU_SC2_BIO"
    for o in dve_ops.OPS:
        if o.name == name:
            return o
    opcode = max(dve_ops._SUB_OPCODE_FOR_NAME.values()) + 1
    assert opcode < 0x20
    dve_ops._SUB_OPCODE_FOR_NAME[name] = opcode

    def _ref(in0, in1, c0, c1, c2):
        a = in0.astype(np.float32).reshape(in0.shape[0], -1)
        b = in1.astype(np.float32).reshape(in1.shape[0], -1)
        s = np.maximum(np.nan_to_num(a * c0 + b * c1, nan=0.0, posinf=np.inf,
                                     neginf=-np.inf), 0)
        return s.reshape(in0.shape)

    spec = Spec(body=_relu_expr(Src0 * C0 + Src1 * C1), reference=_ref)
    shas = {}
    for ver in ("v3", "v4"):
        s = DveOpSpec(name=name, opcode=opcode, uops=lower(spec, ver=ver),
                      rd1_en=True)
        shas[ver] = s.sha(ver)
    op = dve_ops.DveOp(name, spec, subdim=False, uops_sha=shas)
    dve_ops.OPS.append(op)
    dve_ops.CUSTOM_DVE_SPECS[name] = spec
    return op


RELU_SC2 = _register_relu_sc2()

F32 = mybir.dt.float32
F16 = mybir.dt.float16
AOP = mybir.AluOpType

B = 8            # batch per core
R = 512          # n_rec
NIN = 128        # n_in
RC = 4           # r chunks (contraction k and output m)
SUP = RC * B     # 32 ring cols per step
N_CORES = 8
ALPHA = 0.2
LEAK = 1.0 - ALPHA
K = 32           # rescale block (must divide OBLK)


def _drain_mats(OBLK=128):
    """Constant decay matrices for the drain (host-side, fp16).

    ring holds rt_t = r_t / s(i) with s(i) = 0.8^{(i%K)+1}, except
    i%K==K-1 where the boundary relu emits unit scale (s=1).
    h(t0+j) = sum_{i<=j} 0.8^{j-i} s(i) rt_i  +  sum_prev 0.8^{j+128-i} s(i) rt_i
    """
    i_idx = np.arange(OBLK)
    s_i = np.where(i_idx % K == K - 1, 1.0, LEAK ** ((i_idx % K) + 1.0))
    dcur = np.zeros((OBLK, OBLK), np.float32)
    for j in range(OBLK):
        lag = j - i_idx[: j + 1]
        dcur[: j + 1, j] = (LEAK ** lag) * s_i[: j + 1]
    dtail = (LEAK ** (i_idx[None, :] + OBLK - i_idx[:, None])) * s_i[:, None]
    return dcur.astype(np.float16), dtail.astype(np.float16)


def build_nc(T=1000, U=256, use_bacc=True):
    """Build the per-core Bass program. U = ring steps."""
    OBLK = 128
    assert U == 2 * OBLK and OBLK % K == 0
    nc = bacc.Bacc() if use_bacc else bass.Bass()

    x_d = nc.dram_tensor("x_c", [B, T, NIN], F32, kind="ExternalInput").ap()
    n_d = nc.dram_tensor("noise_c", [B, T, R], F32, kind="ExternalInput").ap()
    w_d = nc.dram_tensor("w16", [R, R], F16, kind="ExternalInput").ap()
    wi_d = nc.dram_tensor("win16", [NIN, R], F16, kind="ExternalInput").ap()
    dc_d = nc.dram_tensor("dcur16", [OBLK, OBLK], F16, kind="ExternalInput").ap()
    dt_d = nc.dram_tensor("dtail16", [OBLK, OBLK], F16, kind="ExternalInput").ap()
    o_d = nc.dram_tensor("out_c", [B, T, R], F32, kind="ExternalOutput").ap()

    ZB = 64  # zbuf steps per prepass matmul

    c1 = ALPHA / LEAK                 # steady-state relu C0
    c1_bd = ALPHA * LEAK ** (K - 1)   # boundary relu C0
    c2_bd = LEAK ** K                 # boundary relu C1 (z term); also S scale

    with tile.TileContext(nc) as tc, ExitStack() as ctx:
        const = ctx.enter_context(tc.tile_pool(name="const", bufs=1))
        big = ctx.enter_context(tc.tile_pool(name="big", bufs=1))
        dram = ctx.enter_context(tc.tile_pool(name="dram", bufs=1, space="DRAM"))

        # ---- constants ----
        ident16 = const.tile([128, 128], F16)
        make_identity(nc, ident16[:, :])

        w16 = const.tile([128, RC * R], F16)
        nc.sync.dma_start(
            out=w16[:, :].rearrange("p (k m) -> p k m", m=R),
            in_=w_d.rearrange("(k p) m -> p k m", p=128),
        )
        win16 = const.tile([128, R], F16)
        nc.sync.dma_start(out=win16[:, :], in_=wi_d)
        dcur16 = const.tile([128, OBLK], F16)
        nc.sync.dma_start(out=dcur16[:, :], in_=dc_d)
        dtail16 = const.tile([128, OBLK], F16)
        nc.sync.dma_start(out=dtail16[:, :], in_=dt_d)

        zero16 = const.tile([128, SUP], F16)
        nc.vector.memset(zero16[:, :], 0.0)

        # ---- big persistent buffers ----
        # zbuf m-major planes: col = m*(T*B) + t*B + b   (holds scaled zt)
        zbuf = big.tile([128, RC * T * B], F16)
        xT16 = big.tile([128, T * B], F16)
        # rt ring: col = (t%U)*SUP + k*8 + b
        ring = big.tile([128, U * SUP], F16)
        # transposed-rt staging per (parity, b, m): fp16 (t-part, r-cols)
        sTb = big.tile([128, 2 * SUP * 128], F16)

        zv = zbuf[:, :].rearrange("p (m t b) -> p m t b", t=T, b=B)
        rv = ring[:, :].rearrange("p (t k b) -> p t k b", k=RC, b=B)
        sv = sTb[:, :].rearrange("p (par u c) -> p par u c", par=2, c=128)

        # ---- prepass: DMA cast+reorder to (t,b,r) scratch, then xbar ----
        nscr = dram.tile([T * B, R], F16)
        xscr = dram.tile([T * B, NIN], F16)
        nv = nscr[:, :].rearrange("(t b) r -> t b r", b=B)
        xv_s = xscr[:, :].rearrange("(t b) r -> t b r", b=B)
        ps_z = ctx.enter_context(tc.tile_pool(name="psz", bufs=2, space="PSUM"))
        PIECES = [(0, min(128, T))]
        if T > 128:
            PIECES.append((128, min(448, T)))
        if T > 448:
            PIECES.append((448, T))
        for (t0, t1) in PIECES:
            for b in range(B):
                nc.gpsimd.dma_start(out=nv[t0:t1, b, :], in_=n_d[b, t0:t1, :])
                nc.gpsimd.dma_start(out=xv_s[t0:t1, b, :], in_=x_d[b, t0:t1, :])
        for (t0, t1) in PIECES:
            for m in range(RC):
                nc.sync.dma_start(
                    out=zv[:, m, t0:t1, :].rearrange("p t b -> p (t b)"),
                    in_=nscr[t0 * B:t1 * B, m * 128:(m + 1) * 128],
                    transpose=True,
                )
            nc.sync.dma_start(out=xT16[:, t0 * B:t1 * B],
                              in_=xscr[t0 * B:t1 * B, :], transpose=True)

        def zmm_unit(z0, nt, m):
            # zbuf[m, z0:z0+nt, :] += x~ @ w_in[:, m-chunk]
            zps = ps_z.tile([128, ZB * B], F32, tag="zps")
            nc.tensor.matmul(
                zps[:, :nt * B],
                lhsT=win16[:, m * 128:(m + 1) * 128],
                rhs=xT16[:, z0 * B:(z0 + nt) * B],
                start=True, stop=True,
            )
            zsl = zv[:, m, z0:z0 + nt, :]
            nc.vector.scalar_tensor_tensor(
                out=zsl,
                in0=zps[:, :nt * B].rearrange("p (t b) -> p t b", b=B),
                scalar=0.0, in1=zsl,
                op0=AOP.add, op1=AOP.add,
            )

        def piece_zmm_units(p0, p1):
            return [(z0, min(ZB, p1 - z0), m)
                    for z0 in range(p0, p1, ZB) for m in range(RC)]

        # ---- recurrence + interleaved drain ----
        with tc.tile_pool(name="psq", bufs=1, space="PSUM") as ps_q, \
             tc.tile_pool(name="psot", bufs=2, space="PSUM") as ps_ot, \
             tc.tile_pool(name="psd", bufs=2, space="PSUM") as ps_d, \
             tc.tile_pool(name="sbp", bufs=2) as sbp, \
             tc.tile_pool(name="ostg", bufs=3) as ostg:
            # Q: one full psum bank (start=True resets whole 2KB zero region)
            Q = ps_q.tile([128, 512], F32, name="psq", tag="psq")
            qv = Q[:, 0:SUP].rearrange("p (k b) -> p k b", b=B)

            # prime Q = 0 (cols 0:32)
            for m in range(RC):
                nc.tensor.matmul(Q[:, m * B:(m + 1) * B], lhsT=ident16[:, :],
                                 rhs=zero16[:, m * B:(m + 1) * B],
                                 start=(m == 0), stop=True,
                                 skip_group_check=True)

            pending = []          # drain units
            zqueue = []           # prepass zmm units
            for u in piece_zmm_units(*PIECES[0]):
                zmm_unit(*u)

            def emit_drain_unit(u):
                blk, blk_t0, nt, b, m = u
                rt0 = blk_t0 % U
                par = blk % 2
                ui = b * RC + m
                tp = ps_ot.tile([128, 128], F16, tag="otp")
                nc.tensor.transpose(tp[:nt, :128], rv[:, rt0:rt0 + nt, m, b],
                                    ident16[:, :])
                nc.scalar.copy(out=sv[:nt, par, ui, :], in_=tp[:nt, :128])
                dp = ps_d.tile([128, 128], F32, tag="dps")
                if blk > 0:
                    nc.tensor.matmul(dp[:nt, :], lhsT=dtail16[:, :nt],
                                     rhs=sv[:, 1 - par, ui, :],
                                     start=True, stop=False,
                                     skip_group_check=True)
                nc.tensor.matmul(dp[:nt, :], lhsT=dcur16[:nt, :nt],
                                 rhs=sv[:nt, par, ui, :],
                                 start=(blk == 0), stop=True,
                                 skip_group_check=True)
                st = ostg.tile([128, 128], F32, tag="ost")
                nc.scalar.copy(out=st[:nt, :], in_=dp[:nt, :])
                nc.sync.dma_start(
                    out=o_d[b, blk_t0:blk_t0 + nt, m * 128:(m + 1) * 128],
                    in_=st[:nt, :],
                )

            prev_rd = None
            for t in range(T):
                for pi in range(1, len(PIECES)):
                    if t == PIECES[pi][0] - 80:
                        zqueue.extend(piece_zmm_units(*PIECES[pi]))
                rd = (t - 1) % U
                wr = t % U
                if t > 0:
                    def kmm(k, m):
                        return nc.tensor.matmul(
                            Q[:, m * B:(m + 1) * B],
                            lhsT=w16[:, k * R + m * 128:k * R + (m + 1) * 128],
                            rhs=rv[:, rd, k, :],
                            start=False, stop=True, skip_group_check=True,
                        )
                    for k in (0, 1):          # uses rt-a(t-1)
                        for m in range(RC):
                            kmm(k, m)
                    for k in (2, 3):          # psA closers, use rt-b(t-1)
                        kmm(k, 0)
                        kmm(k, 1)
                    for k in (2, 3):          # psC closers
                        kmm(k, 2)
                        kmm(k, 3)

                j = t % K
                bd = (j == K - 1)
                s0 = c1_bd if bd else c1
                s1 = c2_bd if bd else 1.0
                nc.vector._custom_dve(
                    RELU_SC2,
                    out=rv[:, wr, 0:2, :], in0=qv[:, 0:2, :],
                    in1=zv[:, 0:2, t, :], s0=s0, s1=s1)
                nc.vector._custom_dve(
                    RELU_SC2,
                    out=rv[:, wr, 2:4, :], in0=qv[:, 2:4, :],
                    in1=zv[:, 2:4, t, :], s0=s0, s1=s1)

                if bd and t < T - 1:
                    # rescale psum: S = 0.8^K * Q, re-inject (resets bank)
                    s16 = sbp.tile([128, SUP], F16, tag="s16")
                    nc.scalar.mul(out=s16[:, :], in_=Q[:, 0:SUP], mul=c2_bd)
                    for m in range(RC):
                        nc.tensor.matmul(Q[:, m * B:(m + 1) * B],
                                         lhsT=ident16[:, :],
                                         rhs=s16[:, m * B:(m + 1) * B],
                                         start=(m == 0), stop=True,
                                         skip_group_check=True)

                # drain + prepass fillers (PE work during relu round trip)
                if pending and t % 2 == 0:
                    emit_drain_unit(pending.pop(0))
                elif zqueue:
                    zmm_unit(*zqueue.pop(0))

                if (t + 1) % OBLK == 0 or t == T - 1:
                    blk_t0 = (t // OBLK) * OBLK
                    blk = t // OBLK
                    for b in range(B):
                        for m in range(RC):
                            pending.append((blk, blk_t0, t + 1 - blk_t0, b, m))
            while zqueue:
                zmm_unit(*zqueue.pop(0))
            while pending:
                emit_drain_unit(pending.pop(0))

    if use_bacc:
        nc.compile()
    return nc


def host_prep(x, w_in, w_rec, b_rec, ei_mask, autapse_mask, noise):
    """Host-side weight prep + batch shard. Returns list of per-core in_maps."""
    ei = np.diagonal(np.asarray(ei_mask)).astype(np.float32)
    w_eff = ei[:, None] * (np.asarray(w_rec) * np.asarray(autapse_mask))
    w16 = w_eff.astype(np.float16)
    win16 = np.asarray(w_in).astype(np.float16)
    x = np.asarray(x, dtype=np.float32)
    noise = np.asarray(noise, dtype=np.float32) \
        + np.asarray(b_rec, dtype=np.float32)[None, None, :]
    T = x.shape[1]
    # per-step scale c2(j) = alpha * 0.8^{-(j%K+1)} folded into x and noise
    c2 = (ALPHA * LEAK ** (-((np.arange(T) % K) + 1.0))).astype(np.float32)
    x = x * c2[None, :, None]
    noise = noise * c2[None, :, None]
    dcur16, dtail16 = _drain_mats()
    bs = x.shape[0] // N_CORES
    in_maps = []
    for c in range(N_CORES):
        in_maps.append({
            "x_c": np.ascontiguousarray(x[c * bs:(c + 1) * bs]),
            "noise_c": np.ascontiguousarray(noise[c * bs:(c + 1) * bs]),
            "w16": w16,
            "win16": win16,
            "dcur16": dcur16,
            "dtail16": dtail16,
        })
    return in_maps, w_eff.astype(np.float32)


def reference_np(x, w_in, b_rec, w_eff, noise, T=None):
    """Numpy reference for dev checks (f32). Takes UNSCALED inputs."""
    x = np.asarray(x, np.float32)
    if T is None:
        T = x.shape[1]
    z = np.einsum("bti,ir->btr", x[:, :T], np.asarray(w_in)) \
        + np.asarray(noise)[:, :T] + np.asarray(b_rec)
    h = np.zeros((x.shape[0], w_eff.shape[0]), np.float32)
    outs = []
    for t in range(T):
        pre = z[:, t] + h @ w_eff
        h = LEAK * h + ALPHA * np.maximum(pre, 0.0)
        outs.append(h.copy())
    return np.stack(outs, axis=1)


# ---------------------------------------------------------------------------
# harness entry point
# ---------------------------------------------------------------------------
_NC_CACHE = {}


def kernel(x, w_in, w_rec, b_rec, ei_mask, autapse_mask, noise):
    from concourse.bass_utils import run_bass_kernel_spmd

    x = np.asarray(x)
    T = x.shape[1]
    in_maps, _ = host_prep(x, w_in, w_rec, b_rec, ei_mask, autapse_mask, noise)
    if T not in _NC_CACHE:
        _NC_CACHE[T] = build_nc(T=T)
    nc = _NC_CACHE[T]
    res = run_bass_kernel_spmd(nc, in_maps, core_ids=list(range(N_CORES)))
    out = np.concatenate([r["out_c"] for r in res.results], axis=0)
    return out.astype(np.float32)


# revision 5
# speedup vs baseline: 1.3589x; 1.3589x over previous
"""BioRNN Trainium2 kernel — v3 (rescaled psum + decay-matrix drain,
false-dependency-free tiling).

Math identical to v2 (see numsim.py):
  ring holds rt_t = r_t / s(j), psum accumulates Q_j = Q_{j-1} + rt@W in
  place per K=32-step block; drain reconstructs h per 128-step block as
  h = Dcur^T @ rtT_cur + Dtail^T @ rtT_prev  on the PE.

Tiling for the Tile framework's PER-TILE dependency tracking:
  * QA (m-chunks 0,1) and QC (m-chunks 2,3) in separate psum banks, so
    relu-a gates only on QA's 8 writers and next-step k01 matmuls gate
    only on relu-a.
  * ring split into 4 tiles: (a|b half) x (128-step block parity), so
    drain transposes of block i read a tile no live relu writes.
  * zbuf split per DMA piece; the z-assembly STT runs on Pool (gpsimd),
    keeping DVE exclusively for the two critical-path relus.
  * drain pipelined: transpose+ACT-copy at one tick, D-matmuls+copy+DMA
    two steps later, so the in-order PE never waits on the ACT copy.

Steady-state per step: PE 16 matmuls, DVE 2 relus. Critical cycle:
relu-b -> 4 QA-closer matmuls -> psum commit -> relu-a -> ... (~740ns).
"""

import numpy as np
from contextlib import ExitStack

import concourse.bass as bass
import concourse.mybir as mybir
import concourse.tile as tile
from concourse import bacc
from concourse import dve_ops
from concourse.dve_spec import Spec, Src0, Src1, C0, C1, relu as _relu_expr, lower
from concourse.dve_uop import DveOpSpec
from concourse.masks import make_identity


def _register_relu_sc2():
    """Register fused out = relu(in0*c0 + in1*c1) custom DVE op (idempotent)."""
    name = "RELU_SC2_BIO"
    for o in dve_ops.OPS:
        if o.name == name:
            return o
    opcode = max(dve_ops._SUB_OPCODE_FOR_NAME.values()) + 1
    assert opcode < 0x20
    dve_ops._SUB_OPCODE_FOR_NAME[name] = opcode

    def _ref(in0, in1, c0, c1, c2):
        a = in0.astype(np.float32).reshape(in0.shape[0], -1)
        b = in1.astype(np.float32).reshape(in1.shape[0], -1)
        s = np.maximum(np.nan_to_num(a * c0 + b * c1, nan=0.0, posinf=np.inf,
                                     neginf=-np.inf), 0)
        return s.reshape(in0.shape)

    spec = Spec(body=_relu_expr(Src0 * C0 + Src1 * C1), reference=_ref)
    shas = {}
    for ver in ("v3", "v4"):
        s = DveOpSpec(name=name, opcode=opcode, uops=lower(spec, ver=ver),
                      rd1_en=True)
        shas[ver] = s.sha(ver)
    op = dve_ops.DveOp(name, spec, subdim=False, uops_sha=shas)
    dve_ops.OPS.append(op)
    dve_ops.CUSTOM_DVE_SPECS[name] = spec
    return op


RELU_SC2 = _register_relu_sc2()

F32 = mybir.dt.float32
F16 = mybir.dt.float16
AOP = mybir.AluOpType

B = 8            # batch per core
R = 512          # n_rec
NIN = 128        # n_in
RC = 4           # r chunks (contraction k and output m)
HSUP = 2 * B     # 16 ring cols per step per half
N_CORES = 8
ALPHA = 0.2
LEAK = 1.0 - ALPHA
K = 32           # rescale block (must divide OBLK)
OBLK = 128       # drain block


def _drain_mats():
    """Constant decay matrices for the drain (host-side, fp16)."""
    i_idx = np.arange(OBLK)
    s_i = np.where(i_idx % K == K - 1, 1.0, LEAK ** ((i_idx % K) + 1.0))
    dcur = np.zeros((OBLK, OBLK), np.float32)
    for j in range(OBLK):
        lag = j - i_idx[: j + 1]
        dcur[: j + 1, j] = (LEAK ** lag) * s_i[: j + 1]
    dtail = (LEAK ** (i_idx[None, :] + OBLK - i_idx[:, None])) * s_i[:, None]
    return dcur.astype(np.float16), dtail.astype(np.float16)


def build_nc(T=1000, use_bacc=True):
    """Build the per-core Bass program."""
    nc = bacc.Bacc() if use_bacc else bass.Bass()

    x_d = nc.dram_tensor("x_c", [B, T, NIN], F32, kind="ExternalInput").ap()
    n_d = nc.dram_tensor("noise_c", [B, T, R], F32, kind="ExternalInput").ap()
    w_d = nc.dram_tensor("w16", [R, R], F16, kind="ExternalInput").ap()
    wi_d = nc.dram_tensor("win16", [NIN, R], F16, kind="ExternalInput").ap()
    dc_d = nc.dram_tensor("dcur16", [OBLK, OBLK], F16, kind="ExternalInput").ap()
    dt_d = nc.dram_tensor("dtail16", [OBLK, OBLK], F16, kind="ExternalInput").ap()
    o_d = nc.dram_tensor("out_c", [B, T, R], F32, kind="ExternalOutput").ap()

    ZB = 64  # zbuf steps per prepass matmul

    c1 = ALPHA / LEAK                 # steady-state relu C0
    c1_bd = ALPHA * LEAK ** (K - 1)   # boundary relu C0
    c2_bd = LEAK ** K                 # boundary relu C1; also psum S scale

    PIECES = [(0, min(128, T))]
    if T > 128:
        PIECES.append((128, min(448, T)))
    if T > 448:
        PIECES.append((448, T))

    def piece_of(t):
        for pi, (t0, t1) in enumerate(PIECES):
            if t < t1:
                return pi
        raise AssertionError

    with tile.TileContext(nc) as tc, ExitStack() as ctx:
        const = ctx.enter_context(tc.tile_pool(name="const", bufs=1))
        big = ctx.enter_context(tc.tile_pool(name="big", bufs=1))
        dram = ctx.enter_context(tc.tile_pool(name="dram", bufs=1, space="DRAM"))

        # ---- constants ----
        ident16 = const.tile([128, 128], F16)
        make_identity(nc, ident16[:, :])

        w16 = const.tile([128, RC * R], F16)
        nc.sync.dma_start(
            out=w16[:, :].rearrange("p (k m) -> p k m", m=R),
            in_=w_d.rearrange("(k p) m -> p k m", p=128),
        )
        win16 = const.tile([128, R], F16)
        nc.sync.dma_start(out=win16[:, :], in_=wi_d)
        dcur16 = const.tile([128, OBLK], F16)
        nc.sync.dma_start(out=dcur16[:, :], in_=dc_d)
        dtail16 = const.tile([128, OBLK], F16)
        nc.sync.dma_start(out=dtail16[:, :], in_=dt_d)

        zero16 = const.tile([128, 2 * HSUP], F16)
        nc.vector.memset(zero16[:, :], 0.0)

        # ---- big persistent buffers ----
        # zbuf per piece, m-major planes: col = m*(len*B) + (t-t0)*B + b
        zpieces = [big.tile([128, RC * (t1 - t0) * B], F16,
                            name=f"zp{pi}")
                   for pi, (t0, t1) in enumerate(PIECES)]
        zviews = [zp[:, :].rearrange("p (m t b) -> p m t b",
                                     t=(t1 - t0), b=B)
                  for zp, (t0, t1) in zip(zpieces, PIECES)]
        xT16 = big.tile([128, T * B], F16)
        # rt ring: 4 tiles (half a: k01 / half c: k23) x block parity.
        # col = (t%OBLK)*HSUP + k2*8 + b
        ring_a = [big.tile([128, OBLK * HSUP], F16, name=f"ra{i}")
                  for i in range(2)]
        ring_c = [big.tile([128, OBLK * HSUP], F16, name=f"rc{i}")
                  for i in range(2)]
        rva = [r[:, :].rearrange("p (t k b) -> p t k b", k=2, b=B)
               for r in ring_a]
        rvc = [r[:, :].rearrange("p (t k b) -> p t k b", k=2, b=B)
               for r in ring_c]
        # transposed-rt staging per (parity, b, m): fp16 (t-part, r-cols)
        sTb = big.tile([128, 2 * 4 * B * 128], F16)
        sv = sTb[:, :].rearrange("p (par u c) -> p par u c", par=2, c=128)

        def zvt(t):
            pi = piece_of(t)
            return zviews[pi], t - PIECES[pi][0]

        # ---- prepass: DMA cast+reorder to (t,b,r) scratch, then xbar ----
        nscr = dram.tile([T * B, R], F16)
        xscr = dram.tile([T * B, NIN], F16)
        nv = nscr[:, :].rearrange("(t b) r -> t b r", b=B)
        xv_s = xscr[:, :].rearrange("(t b) r -> t b r", b=B)
        ps_z = ctx.enter_context(tc.tile_pool(name="psz", bufs=2, space="PSUM"))
        for (t0, t1) in PIECES:
            for b in range(B):
                nc.gpsimd.dma_start(out=nv[t0:t1, b, :], in_=n_d[b, t0:t1, :])
                nc.gpsimd.dma_start(out=xv_s[t0:t1, b, :], in_=x_d[b, t0:t1, :])
        for pi, (t0, t1) in enumerate(PIECES):
            for m in range(RC):
                nc.sync.dma_start(
                    out=zviews[pi][:, m, :, :].rearrange("p t b -> p (t b)"),
                    in_=nscr[t0 * B:t1 * B, m * 128:(m + 1) * 128],
                    transpose=True,
                )
            nc.sync.dma_start(out=xT16[:, t0 * B:t1 * B],
                              in_=xscr[t0 * B:t1 * B, :], transpose=True)

        def zmm_unit(z0, nt, m):
            # zbuf[m, z0:z0+nt, :] = noise (already there) + x~ @ w_in[m].
            # Identity-matmul injects noise into psum, x-proj accumulates,
            # ACT copies back — keeps DVE free for the critical relus.
            zview, tr0 = zvt(z0)
            zsl = zview[:, m, tr0:tr0 + nt, :]
            zps = ps_z.tile([128, ZB * B], F32, tag="zps")
            nc.tensor.matmul(
                zps[:, :nt * B],
                lhsT=ident16[:, :],
                rhs=zsl.rearrange("p t b -> p (t b)"),
                start=True, stop=False,
            )
            nc.tensor.matmul(
                zps[:, :nt * B],
                lhsT=win16[:, m * 128:(m + 1) * 128],
                rhs=xT16[:, z0 * B:(z0 + nt) * B],
                start=False, stop=True,
            )
            nc.scalar.copy(out=zsl,
                           in_=zps[:, :nt * B].rearrange("p (t b) -> p t b",
                                                         b=B))

        def piece_zmm_units(p0, p1):
            return [(z0, min(ZB, p1 - z0), m)
                    for z0 in range(p0, p1, ZB) for m in range(RC)]

        # ---- recurrence + interleaved drain ----
        with tc.tile_pool(name="psqa", bufs=1, space="PSUM") as ps_qa, \
             tc.tile_pool(name="psqc", bufs=1, space="PSUM") as ps_qc, \
             tc.tile_pool(name="psot", bufs=2, space="PSUM") as ps_ot, \
             tc.tile_pool(name="psd", bufs=2, space="PSUM") as ps_d, \
             tc.tile_pool(name="sbp", bufs=2) as sbp, \
             tc.tile_pool(name="ostg", bufs=3) as ostg:
            QA = ps_qa.tile([128, 512], F32, name="psqa", tag="psqa")
            QC = ps_qc.tile([128, 512], F32, name="psqc", tag="psqc")
            qav = QA[:, 0:HSUP].rearrange("p (k b) -> p k b", b=B)
            qcv = QC[:, 0:HSUP].rearrange("p (k b) -> p k b", b=B)

            # prime QA/QC = 0 (cols 0:16)
            for Qx in (QA, QC):
                for m in range(2):
                    nc.tensor.matmul(Qx[:, m * B:(m + 1) * B],
                                     lhsT=ident16[:, :],
                                     rhs=zero16[:, m * B:(m + 1) * B],
                                     start=(m == 0), stop=True,
                                     skip_group_check=True)

            pending1 = []         # drain stage-1 units
            pending2 = []         # drain stage-2 units
            zqueue = []           # prepass zmm units
            for u in piece_zmm_units(*PIECES[0]):
                zmm_unit(*u)

            def drain_stage1(u):
                blk, blk_t0, nt, b, m = u
                par = blk % 2
                rsrc = rva[par] if m < 2 else rvc[par]
                tp = ps_ot.tile([128, 128], F16, tag="otp")
                nc.tensor.transpose(tp[:nt, :128], rsrc[:, 0:nt, m % 2, b],
                                    ident16[:, :])
                nc.scalar.copy(out=sv[:nt, par, b * RC + m, :],
                               in_=tp[:nt, :128])

            def drain_stage2(u):
                blk, blk_t0, nt, b, m = u
                par = blk % 2
                ui = b * RC + m
                dp = ps_d.tile([128, 128], F32, tag="dps")
                if blk > 0:
                    nc.tensor.matmul(dp[:nt, :], lhsT=dtail16[:, :nt],
                                     rhs=sv[:, 1 - par, ui, :],
                                     start=True, stop=False,
                                     skip_group_check=True)
                nc.tensor.matmul(dp[:nt, :], lhsT=dcur16[:nt, :nt],
                                 rhs=sv[:nt, par, ui, :],
                                 start=(blk == 0), stop=True,
                                 skip_group_check=True)
                st = ostg.tile([128, 128], F32, tag="ost")
                nc.scalar.copy(out=st[:nt, :], in_=dp[:nt, :])
                nc.sync.dma_start(
                    out=o_d[b, blk_t0:blk_t0 + nt, m * 128:(m + 1) * 128],
                    in_=st[:nt, :],
                )

            for t in range(T):
                for pi in range(1, len(PIECES)):
                    if t == PIECES[pi][0] - 80:
                        zqueue.extend(piece_zmm_units(*PIECES[pi]))
                wr = t % OBLK
                wpar = (t // OBLK) % 2
                if t > 0:
                    rd = (t - 1) % OBLK
                    rpar = ((t - 1) // OBLK) % 2

                    def kmm(k, m):
                        rsrc = rva[rpar] if k < 2 else rvc[rpar]
                        Qx = QA if m < 2 else QC
                        return nc.tensor.matmul(
                            Qx[:, (m % 2) * B:(m % 2 + 1) * B],
                            lhsT=w16[:, k * R + m * 128:k * R + (m + 1) * 128],
                            rhs=rsrc[:, rd, k % 2, :],
                            start=False, stop=True, skip_group_check=True,
                        )
                    for k in (0, 1):          # rt-a gated
                        kmm(k, 0)
                        kmm(k, 1)
                    for k in (2, 3):          # QA closers (rt-b gated)
                        kmm(k, 0)
                        kmm(k, 1)
                    for k in (0, 1):          # QC, rt-a gated
                        kmm(k, 2)
                        kmm(k, 3)
                    for k in (2, 3):          # QC closers
                        kmm(k, 2)
                        kmm(k, 3)

                j = t % K
                bd = (j == K - 1)
                s0 = c1_bd if bd else c1
                s1 = c2_bd if bd else 1.0
                zview, tr0 = zvt(t)
                nc.vector._custom_dve(
                    RELU_SC2,
                    out=rva[wpar][:, wr, :, :], in0=qav[:, :, :],
                    in1=zview[:, 0:2, tr0, :], s0=s0, s1=s1)
                nc.vector._custom_dve(
                    RELU_SC2,
                    out=rvc[wpar][:, wr, :, :], in0=qcv[:, :, :],
                    in1=zview[:, 2:4, tr0, :], s0=s0, s1=s1)

                if bd and t < T - 1:
                    # rescale psum: S = 0.8^K * Q, re-inject (resets banks)
                    s16a = sbp.tile([128, HSUP], F16, tag="s16a")
                    s16b = sbp.tile([128, HSUP], F16, tag="s16b")
                    nc.scalar.mul(out=s16a[:, :], in_=QA[:, 0:HSUP],
                                  mul=c2_bd)
                    nc.scalar.mul(out=s16b[:, :], in_=QC[:, 0:HSUP],
                                  mul=c2_bd)
                    for Qx, sx in ((QA, s16a), (QC, s16b)):
                        for m in range(2):
                            nc.tensor.matmul(
                                Qx[:, m * B:(m + 1) * B],
                                lhsT=ident16[:, :],
                                rhs=sx[:, m * B:(m + 1) * B],
                                start=(m == 0), stop=True,
                                skip_group_check=True)

                # drain + prepass fillers (PE work during relu round trip)
                if t % 2 == 0:
                    if pending2:
                        drain_stage2(pending2.pop(0))
                    if pending1:
                        u = pending1.pop(0)
                        drain_stage1(u)
                        pending2.append(u)
                    elif zqueue:
                        zmm_unit(*zqueue.pop(0))
                elif zqueue:
                    zmm_unit(*zqueue.pop(0))

                if (t + 1) % OBLK == 0 or t == T - 1:
                    blk_t0 = (t // OBLK) * OBLK
                    blk = t // OBLK
                    for b in range(B):
                        for m in range(RC):
                            pending1.append((blk, blk_t0, t + 1 - blk_t0, b, m))
            while zqueue:
                zmm_unit(*zqueue.pop(0))
            while pending1:
                u = pending1.pop(0)
                drain_stage1(u)
                pending2.append(u)
            while pending2:
                drain_stage2(pending2.pop(0))

    if use_bacc:
        nc.compile()
    return nc


def host_prep(x, w_in, w_rec, b_rec, ei_mask, autapse_mask, noise):
    """Host-side weight prep + batch shard. Returns list of per-core in_maps."""
    ei = np.diagonal(np.asarray(ei_mask)).astype(np.float32)
    w_eff = ei[:, None] * (np.asarray(w_rec) * np.asarray(autapse_mask))
    w16 = w_eff.astype(np.float16)
    win16 = np.asarray(w_in).astype(np.float16)
    x = np.asarray(x, dtype=np.float32)
    noise = np.asarray(noise, dtype=np.float32) \
        + np.asarray(b_rec, dtype=np.float32)[None, None, :]
    T = x.shape[1]
    c2 = (ALPHA * LEAK ** (-((np.arange(T) % K) + 1.0))).astype(np.float32)
    x = x * c2[None, :, None]
    noise = noise * c2[None, :, None]
    dcur16, dtail16 = _drain_mats()
    bs = x.shape[0] // N_CORES
    in_maps = []
    for c in range(N_CORES):
        in_maps.append({
            "x_c": np.ascontiguousarray(x[c * bs:(c + 1) * bs]),
            "noise_c": np.ascontiguousarray(noise[c * bs:(c + 1) * bs]),
            "w16": w16,
            "win16": win16,
            "dcur16": dcur16,
            "dtail16": dtail16,
        })
    return in_maps, w_eff.astype(np.float32)


def reference_np(x, w_in, b_rec, w_eff, noise, T=None):
    """Numpy reference for dev checks (f32). Takes UNSCALED inputs."""
    x = np.asarray(x, np.float32)
    if T is None:
        T = x.shape[1]
    z = np.einsum("bti,ir->btr", x[:, :T], np.asarray(w_in)) \
        + np.asarray(noise)[:, :T] + np.asarray(b_rec)
    h = np.zeros((x.shape[0], w_eff.shape[0]), np.float32)
    outs = []
    for t in range(T):
        pre = z[:, t] + h @ w_eff
        h = LEAK * h + ALPHA * np.maximum(pre, 0.0)
        outs.append(h.copy())
    return np.stack(outs, axis=1)


# ---------------------------------------------------------------------------
# harness entry point
# ---------------------------------------------------------------------------
_NC_CACHE = {}


def kernel(x, w_in, w_rec, b_rec, ei_mask, autapse_mask, noise):
    from concourse.bass_utils import run_bass_kernel_spmd

    x = np.asarray(x)
    T = x.shape[1]
    in_maps, _ = host_prep(x, w_in, w_rec, b_rec, ei_mask, autapse_mask, noise)
    if T not in _NC_CACHE:
        _NC_CACHE[T] = build_nc(T=T)
    nc = _NC_CACHE[T]
    res = run_bass_kernel_spmd(nc, in_maps, core_ids=list(range(N_CORES)))
    out = np.concatenate([r["out_c"] for r in res.results], axis=0)
    return out.astype(np.float32)
